# revision 34
# baseline (speedup 1.0000x reference)
"""Multi-head self-attention with RoPE on 8 Trainium2 NeuronCores.

Problem: B=2, S=2048, D=1024, H=16 heads, HD=64, causal, fp32.

Sharding: batch x head-group tensor parallel — core c owns batch c//4 and
heads 4*(c%4) .. 4*(c%4)+3 (two head-pairs). Each core computes its heads'
Q/K/V projections, RoPE, causal attention over its batch's 2048 tokens,
and a partial out-projection; the host sums 4 partials per batch and adds
b_out.

Design (engine balance: PE ~111us, ACT ~85us, DVE ~79us busy per rep):
- Phase 1 (QKV+RoPE): 512-token psum chunks, double-buffered (6 banks) +
  rot psum (2), so projection matmuls overlap rope evacuation.  RoPE via
  sin-prescale: t=(x+b)*sinpre, rot(t) by permutation matmul, fx =
  (x+b)*cos + rot — no ACT raw-copies, no shifted-bias tables.
- V: bias folded into the ACT psum->sbuf copy (per-partition bias), then
  token-major transpose via DMA XBAR (one [64,1024] transpose per head),
  then DVE converts to fp8 hi + fp8 residual-lo DoubleRow pair layout.
- Attention is ACT(exp)-bound: per (q-chunk c, head-pair p), slots
  (pair j, head h) stream scores->exp on PE/ACT while the PV matmul is
  emitted PVLAG=2 slots late so the in-order PE queue never head-blocks
  on the exp.  Diagonal pairs run first (their smaller exps absorb the
  chunk-boundary stall); out-projection token-groups are drip-fed into
  the next chunk's slots to fill the PE idle under the exp.
- PV in fp8e4m3 DoubleRow: two 128-token k-blocks contracted per matmul
  ([Ki=128, Ko=2, cols] weights; [128, 2, q] moving); a second DR matmul
  adds e4m3(v - v_hi) (V quantization error -> ~0.1%).  exp is written
  as fp8 exp(s*scale - 2) (softmax-invariant shift keeps e4m3 range).
  [V|ones]/[ones|V] columns carry the softmax denominator in the same
  matmul; ones-columns of the lo tile are zero.
- causal triangle applied pre-exp inside PSUM by an extra N=128 matmul
  (lhsT=trimask upper-triangular -BIG, rhs=eye): exp of masked lanes is
  exactly 0; no DVE triangle multiplies.  Diagonal pairs write into
  persistent exp tiles whose never-rewritten regions are prezeroed once.
"""

import sys

if "/opt/trn_rl_repo" not in sys.path:
    sys.path.insert(0, "/opt/trn_rl_repo")

import numpy as np
import ml_dtypes

import concourse.bass as bass
import concourse.mybir as mybir
import concourse.tile as tile
from concourse import bacc
from concourse.bass_utils import run_bass_kernel_spmd

F32 = mybir.dt.float32
F16 = mybir.dt.float16
BF16 = mybir.dt.bfloat16
F8 = mybir.dt.float8e4
AF = mybir.ActivationFunctionType
ALU = mybir.AluOpType
DR = mybir.MatmulPerfMode.DoubleRow

B, S, D, H, HD = 2, 2048, 1024, 16, 64
NCORES = 8
GPB = NCORES // B              # head-groups per batch = 4
HPC = H // GPB                 # heads per core = 4 (2 pairs)
NP = HPC // 2                  # head pairs per core = 2
CW = HPC * HD                  # feature width per core = 256
ROPE_BASE = 10000.0
SCALE = 1.0 / np.sqrt(HD)
EXP_BIAS = -2.0                # softmax-invariant shift to fit e4m3

_CACHED = {}


def _mm(nc, out, lhsT, rhs, **kw):
    nc.tensor.matmul(out, lhsT, rhs, **kw)


def build_nc(reps=1):
    nc = bacc.Bacc("TRN2", target_bir_lowering=False, debug=False,
                   num_devices=NCORES)

    qT = nc.dram_tensor("qT", [D, S], BF16, kind="ExternalInput")
    wq = nc.dram_tensor("wq", [D, CW], BF16, kind="ExternalInput")
    wk = nc.dram_tensor("wk", [D, CW], BF16, kind="ExternalInput")
    wv = nc.dram_tensor("wv", [D, CW], BF16, kind="ExternalInput")
    bq = nc.dram_tensor("bq", [128, NP], F32, kind="ExternalInput")
    bk = nc.dram_tensor("bk", [128, NP], F32, kind="ExternalInput")
    bv = nc.dram_tensor("bv", [128, NP], F32, kind="ExternalInput")
    cosT = nc.dram_tensor("cosT", [128, S], F16, kind="ExternalInput")
    sinpT = nc.dram_tensor("sinpT", [128, S], F16, kind="ExternalInput")
    trimask = nc.dram_tensor("trimask", [128, 128], BF16, kind="ExternalInput")
    rp = nc.dram_tensor("rp", [128, 128], BF16, kind="ExternalInput")
    eye = nc.dram_tensor("eye", [128, 128], BF16, kind="ExternalInput")
    wout = nc.dram_tensor("wout", [CW, D], BF16, kind="ExternalInput")
    outp = nc.dram_tensor("outp", [S, D], BF16, kind="ExternalOutput")

    KT = D // 128               # 8 contraction tiles

    with tile.TileContext(nc) as tc:
        with (
            tc.tile_pool(name="const", bufs=1) as cpool,
            tc.tile_pool(name="persist", bufs=1) as ppool,
        ):
            # ---- constants resident in SBUF ----
            wq_sb = cpool.tile([128, KT, CW], BF16)
            wk_sb = cpool.tile([128, KT, CW], BF16)
            wv_sb = cpool.tile([128, KT, CW], BF16)
            nc.sync.dma_start(wq_sb[:], wq[:].rearrange("(a p) f -> p a f", p=128))
            nc.sync.dma_start(wk_sb[:], wk[:].rearrange("(a p) f -> p a f", p=128))
            nc.sync.dma_start(wv_sb[:], wv[:].rearrange("(a p) f -> p a f", p=128))
            wout_sb = cpool.tile([128, CW // 128, D], BF16)
            nc.sync.dma_start(wout_sb[:],
                              wout[:].rearrange("(g p) f -> p g f", p=128))
            tm_sb = cpool.tile([128, 128], BF16)
            nc.sync.dma_start(tm_sb[:], trimask[:])
            rp_sb = cpool.tile([128, 128], BF16)
            nc.sync.dma_start(rp_sb[:], rp[:])
            eye_sb = cpool.tile([128, 128], BF16)
            nc.sync.dma_start(eye_sb[:], eye[:])
            bq_sb = cpool.tile([128, NP], F32)
            bk_sb = cpool.tile([128, NP], F32)
            bv_sb = cpool.tile([128, NP], F32)
            nc.sync.dma_start(bq_sb[:], bq[:])
            nc.sync.dma_start(bk_sb[:], bk[:])
            nc.sync.dma_start(bv_sb[:], bv[:])
            cos_sb = cpool.tile([128, S], F16)
            sinp_sb = cpool.tile([128, S], F16)
            nc.sync.dma_start(cos_sb[:], cosT[:])
            nc.sync.dma_start(sinp_sb[:], sinpT[:])
            ebias_sb = cpool.tile([128, 1], F32)
            nc.gpsimd.memset(ebias_sb[:], EXP_BIAS)

            # ---- persistent activations ----
            qf_t = [ppool.tile([128, NP, S // 2], BF16, name=f"qf{t}")
                    for t in range(2)]
            kf_t = [ppool.tile([128, NP, S // 2], BF16, name=f"kf{t}")
                    for t in range(2)]
            # token-major V, fp8 DoubleRow pairs: [tok, pair, ko, 192]
            # cols: [V_hA(64) | ones(64) | V_hB(64)]
            vt8_t = [ppool.tile([128, NP, S // 512, 2, 192], F8, name=f"vt8{t}")
                     for t in range(2)]
            vt8lo_t = [ppool.tile([128, NP, S // 512, 2, 192], F8,
                                  name=f"vt8lo{t}") for t in range(2)]
            # bf16 token-major staging (DMA-transpose target)
            vst_t = [[ppool.tile([128, S // 256, 192], BF16, name=f"vs{t}{p}")
                      for p in range(NP)] for t in range(2)]
            at_sb = ppool.tile([128, NP, S], BF16)   # attn^T, stacked heads

            for t in range(2):
                nc.gpsimd.memset(vt8_t[t][:, :, :, :, 64:128], 1.0)
                nc.gpsimd.memset(vt8lo_t[t][:, :, :, :, 64:128], 0.0)
            # prezeroed diagonal exp tiles (zero regions never rewritten):
            # T1: pair (m0,m1): block B cols [0:128) zero
            # T2: pair (m2,m3): block B cols [256:384) zero (PV starts at 256)
            t1_sb = [ppool.tile([128, 2, 512], F8, name=f"t1_{i}")
                     for i in range(2)]
            t2_sb = [ppool.tile([128, 2, 512], F8, name=f"t2_{i}")
                     for i in range(2)]
            for i in range(2):
                nc.gpsimd.memset(t1_sb[i][:, 1, 0:128], 0.0)
                nc.gpsimd.memset(t2_sb[i][:, 1, 256:384], 0.0)

            for _rep in range(reps):
                _build_body(nc, tc, locals())

    nc.compile()
    return nc


def _build_body(nc, tc, env):
    qT, outp = env["qT"], env["outp"]
    wq_sb, wk_sb, wv_sb = env["wq_sb"], env["wk_sb"], env["wv_sb"]
    wout_sb = env["wout_sb"]
    cos_sb, sinp_sb = env["cos_sb"], env["sinp_sb"]
    tm_sb, rp_sb, eye_sb = env["tm_sb"], env["rp_sb"], env["eye_sb"]
    bq_sb, bk_sb, bv_sb = env["bq_sb"], env["bk_sb"], env["bv_sb"]
    ebias_sb = env["ebias_sb"]
    qf_t, kf_t = env["qf_t"], env["kf_t"]
    vt8_t, vst_t, at_sb = env["vt8_t"], env["vst_t"], env["at_sb"]
    vt8lo_t = env["vt8lo_t"]
    t1_sb, t2_sb = env["t1_sb"], env["t2_sb"]
    KT = env["KT"]

    # =========== phase 1: QKV projection + RoPE + V transpose ===========
    # half-granularity (512-token) psum tiles, double-buffered, so the
    # next chunk's projection matmuls overlap this chunk's rope evacuation
    with (
        tc.tile_pool(name="qt", bufs=12) as qtp,
        tc.tile_pool(name="rt", bufs=6) as rtp,
        tc.tile_pool(name="vf", bufs=3) as vfp,
        tc.tile_pool(name="pmain", bufs=2, space="PSUM") as pmain,
        tc.tile_pool(name="prot", bufs=2, space="PSUM") as protp,
    ):
        for tp in range(2):                      # 1024-token chunks
            tps = slice(1024 * tp, 1024 * (tp + 1))
            qts = []
            for kt in range(KT):
                qt_sb = qtp.tile([128, 1024], BF16, tag="qt", name=f"qt{kt}")
                qts.append(qt_sb)
                nc.sync.dma_start(qt_sb[:], qT[128 * kt:128 * (kt + 1), tps])
            for p in range(NP):                  # head pairs
                pf = slice(128 * p, 128 * (p + 1))
                vf = vfp.tile([128, 1024], BF16, tag="vf", name="vf")
                for i in range(2):               # 512-token halves
                    hs = slice(512 * i, 512 * (i + 1))
                    tps_i = slice(1024 * tp + 512 * i,
                                  1024 * tp + 512 * (i + 1))
                    ps_q = pmain.tile([128, 512], F32, tag="psq", name="psq")
                    ps_k = pmain.tile([128, 512], F32, tag="psk", name="psk")
                    ps_v = pmain.tile([128, 512], F32, tag="psv", name="psv")
                    for kt in range(KT):
                        for w_sb, ps_x in ((wq_sb, ps_q), (wk_sb, ps_k),
                                           (wv_sb, ps_v)):
                            _mm(nc, ps_x[:], w_sb[:, kt, pf], qts[kt][:, hs],
                                start=(kt == 0), stop=(kt == KT - 1))

                    for psx, fx, bx in ((ps_q, qf_t[tp], bq_sb),
                                        (ps_k, kf_t[tp], bk_sb)):
                        # t = (x + b) * sinpre ; rot(t) via permutation mm
                        t_sb = rtp.tile([128, 512], BF16, tag="rt", name="rt")
                        nc.vector.scalar_tensor_tensor(
                            t_sb[:], psx[:], bx[:, p:p + 1],
                            sinp_sb[:, tps_i], ALU.add, ALU.mult)
                        ps_r = protp.tile([128, 512], F32, tag="rot",
                                          name="rot")
                        _mm(nc, ps_r[:], rp_sb[:], t_sb[:],
                            start=True, stop=True)
                        # fx = (x + b) * cos ; fx += rot
                        nc.vector.scalar_tensor_tensor(
                            fx[:, p, hs], psx[:], bx[:, p:p + 1],
                            cos_sb[:, tps_i], ALU.add, ALU.mult)
                        nc.vector.tensor_add(fx[:, p, hs], fx[:, p, hs],
                                             ps_r[:])

                    # V: psum -> sbuf with bias folded into the ACT copy
                    nc.scalar.activation(vf[:, hs], ps_v[:], AF.Identity,
                                         bias=bv_sb[:, p:p + 1])

                # DMA-transpose V to token-major, then fp8 pair layout
                vst = vst_t[tp][p]
                for h in range(2):
                    nc.sync.dma_start(vst[:, :, 128 * h:128 * h + 64],
                                      vf[64 * h:64 * h + 64, :],
                                      transpose=True)
                for h in range(2):
                    src = vst[:, :, 128 * h:128 * h + 64].rearrange(
                        "p (a b) c -> p a b c", b=2)
                    dst = vt8_t[tp][:, p, :, :, 128 * h:128 * h + 64]
                    nc.vector.tensor_copy(dst, src)
                    # fp8 residual: vlo = e4m3(v_bf16 - v_hi)
                    nc.vector.tensor_sub(
                        vt8lo_t[tp][:, p, :, :, 128 * h:128 * h + 64],
                        src, dst)

    # =========== phase 2+3: attention + out-projection ===========
    with (
        tc.tile_pool(name="sps", bufs=2, space="PSUM") as sps,
        tc.tile_pool(name="aps", bufs=2, space="PSUM") as aps,
        tc.tile_pool(name="exppool", bufs=6) as expp,
        tc.tile_pool(name="recip", bufs=4) as rcpp,
        tc.tile_pool(name="ostage", bufs=6) as ostp,
    ):
        def emit_norm(c, p, ph):
            # normalize: attnT = attn_rows * (1 / sum_rows)
            cs = slice(512 * c, 512 * (c + 1))
            rc = rcpp.tile([128, 512], F32, tag="rc", name="rc")
            nc.vector.reciprocal(rc[0:64, :], ph[0][64:128, :])
            nc.vector.reciprocal(rc[64:128, :], ph[1][0:64, :])
            nc.vector.tensor_mul(at_sb[0:64, p, cs],
                                 ph[0][0:64, :], rc[0:64, :])
            nc.vector.tensor_mul(at_sb[64:128, p, cs],
                                 ph[1][64:128, :], rc[64:128, :])

        def emit_outproj_unit(tt, _nf):
            # one 128-token group of the out-projection
            trows = slice(128 * tt, 128 * (tt + 1))
            pso = sps.tile([128, 2, 512], F32, tag="ps_s", name="ps_o")
            for nf in range(2):
                fs = slice(512 * nf, 512 * (nf + 1))
                for p in range(NP):
                    _mm(nc, pso[:, nf, :], at_sb[:, p, trows],
                        wout_sb[:, p, fs],
                        start=(p == 0), stop=(p == NP - 1))
            o_sb = ostp.tile([128, 1024], BF16, tag="ost", name="ost")
            nc.vector.tensor_copy(o_sb[:],
                                  pso[:].rearrange("p a b -> p (a b)"))
            nc.sync.dma_start(outp[trows, :], o_sb[:])

        # software pipeline: PV runs PVLAG slots behind scores/exp so the
        # in-order PE queue never head-blocks on the ACT exp; out-proj
        # token groups are drip-fed into the next chunk's slot stream so
        # they hide in the PE idle under the ACT-bound exp.
        PVLAG = 2
        pvq = []
        outq = []

        def pop_pv():
            (c, p, j, h, ph, dexp, o, first, last) = pvq.pop(0)
            # PV DoubleRow: hA lhsT=[V|ones], hB lhsT=[ones|V];
            # second matmul adds the fp8 V-residual (zero ones-cols)
            _mm(nc, ph[h][:, o:512],
                vt8_t[j // 4][:, p, j % 4, :, 64 * h:64 * h + 128],
                dexp[:, :, o:512],
                start=first, stop=False, perf_mode=DR)
            _mm(nc, ph[h][:, o:512],
                vt8lo_t[j // 4][:, p, j % 4, :, 64 * h:64 * h + 128],
                dexp[:, :, o:512],
                start=False, stop=last, perf_mode=DR)
            if last and h == 1:
                emit_norm(c, p, ph)
                if p == NP - 1:
                    outq.extend((tt, 0) for tt in range(4 * c, 4 * c + 4))

        nslot = 0
        for c in range(4):
            # diag pairs first: their smaller exps land where the (c,p)
            # boundary stalls anyway; D1 (full-width) must stay the
            # accumulation opener.
            order = [2 * c, 2 * c + 1] + list(range(2 * c))
            for p in range(NP):
                ph = [aps.tile([128, 512], F32, tag=f"pa{h}", name=f"pa{h}")
                      for h in range(2)]
                for jx, j in enumerate(order):
                    diag = j >= 2 * c
                    d2 = j == 2 * c + 1
                    first = jx == 0
                    last = jx == 2 * c + 1
                    for h in range(2):
                        p0 = 64 * h
                        ps_s = sps.tile([128, 2, 512], F32, tag="ps_s",
                                        name="ps_s")
                        if diag:
                            # per-block exp right after each block's mask,
                            # so ACT starts while the other block's scores
                            # are still on the PE
                            dexp = t1_sb[h] if not d2 else t2_sb[h]
                            o = 0 if not d2 else 256
                            exp_lo = (0, 128) if not d2 else (256, 384)
                        else:
                            dexp = None
                        for jj in range(2):
                            r = 2 * j + jj
                            ks_ = slice(128 * (r % 8), 128 * (r % 8) + 128)
                            m = r - 4 * c
                            mm_ = max(m, 0)
                            qs_ = slice(512 * (c % 2) + 128 * mm_,
                                        512 * (c % 2) + 512)
                            _mm(nc, ps_s[:, jj, 128 * mm_:512],
                                kf_t[r // 8][p0:p0 + 64, p, ks_],
                                qf_t[c // 2][p0:p0 + 64, p, qs_],
                                start=True, stop=(m < 0))
                            if m >= 0:
                                # -BIG upper triangle onto the diag window
                                _mm(nc, ps_s[:, jj, 128 * m:128 * m + 128],
                                    tm_sb[:], eye_sb[:],
                                    start=False, stop=True)
                                nc.scalar.activation(
                                    dexp[:, jj, exp_lo[jj]:512],
                                    ps_s[:, jj, exp_lo[jj]:512],
                                    AF.Exp, bias=ebias_sb[:, 0:1],
                                    scale=float(SCALE))
                        # exp -> fp8 (shifted) for full pairs in one instr
                        if not diag:
                            dexp = expp.tile([128, 2, 512], F8, tag="exp",
                                             name="exp")
                            nc.scalar.activation(
                                dexp[:].rearrange("p a b -> p (a b)"),
                                ps_s[:].rearrange("p a b -> p (a b)"),
                                AF.Exp, bias=ebias_sb[:, 0:1], scale=float(SCALE))
                            o = 0
                        pvq.append((c, p, j, h, ph, dexp, o, first, last))
                        if len(pvq) > PVLAG:
                            pop_pv()
                        nslot += 1
                        if outq and nslot % 3 == 0:
                            emit_outproj_unit(*outq.pop(0))
        while pvq:
            pop_pv()
        while outq:
            emit_outproj_unit(*outq.pop(0))


def _host_prep(query, W_qkv, b_qkv, W_out, b_out):
    """Build per-core input maps. Core c: batch c//GPB, head-group c%GPB."""
    query = np.asarray(query, dtype=np.float32)
    qTb = [np.ascontiguousarray(query[b].T) for b in range(B)]  # (D, S)

    inv_freq = 1.0 / (ROPE_BASE ** (np.arange(0, HD, 2, dtype=np.float32) / HD))
    freqs = np.arange(S, dtype=np.float32)[:, None] * inv_freq[None, :]
    emb = np.concatenate([freqs, freqs], axis=-1)          # (S, 64)
    cos = np.cos(emb).astype(np.float32).T                  # (64, S)
    sin = np.sin(emb).astype(np.float32).T
    sinp = sin.copy()
    sinp[0:32] = -sin[0:32]                                 # sign-folded
    # sin-prescale table: sinpre[d] = sinp[swap(d)]
    sinpre = np.empty_like(sinp)
    sinpre[0:32] = sinp[32:64]
    sinpre[32:64] = sinp[0:32]
    cos128 = np.ascontiguousarray(np.tile(cos, (2, 1)))     # (128, S)
    sinpre128 = np.ascontiguousarray(np.tile(sinpre, (2, 1)))

    BIG = 30000.0
    trimask = np.zeros((128, 128), dtype=np.float32)        # [q, k] = -BIG, k>q
    trimask[np.arange(128)[:, None] < np.arange(128)[None, :]] = -BIG
    eye = np.eye(128, dtype=np.float32)
    # rotate-half permutation: rp[k, m] = 1 iff k == swap(m)
    rp = np.zeros((128, 128), dtype=np.float32)
    for h in range(2):
        for i in range(64):
            rp[64 * h + (i + 32) % 64, 64 * h + i] = 1.0

    W_qkv = np.asarray(W_qkv, dtype=np.float32)
    b_qkv = np.asarray(b_qkv, dtype=np.float32)
    W_out = np.asarray(W_out, dtype=np.float32)

    in_maps = []
    for c in range(NCORES):
        b = c // GPB
        g = c % GPB
        cols = slice(CW * g, CW * (g + 1))
        bqc = np.ascontiguousarray(b_qkv[0:D][cols].reshape(NP, 128).T)
        bkc = np.ascontiguousarray(b_qkv[D:2 * D][cols].reshape(NP, 128).T)
        bvc = np.ascontiguousarray(b_qkv[2 * D:3 * D][cols].reshape(NP, 128).T)
        in_maps.append({
            "qT": qTb[b].astype(ml_dtypes.bfloat16),
            "wq": np.ascontiguousarray(W_qkv[:, 0:D][:, cols]).astype(ml_dtypes.bfloat16),
            "wk": np.ascontiguousarray(W_qkv[:, D:2 * D][:, cols]).astype(ml_dtypes.bfloat16),
            "wv": np.ascontiguousarray(W_qkv[:, 2 * D:3 * D][:, cols]).astype(ml_dtypes.bfloat16),
            "bq": bqc,
            "bk": bkc,
            "bv": bvc,
            "cosT": cos128.astype(np.float16),
            "sinpT": sinpre128.astype(np.float16),
            "trimask": trimask.astype(ml_dtypes.bfloat16),
            "rp": rp.astype(ml_dtypes.bfloat16),
            "eye": eye.astype(ml_dtypes.bfloat16),
            "wout": np.ascontiguousarray(W_out[CW * g:CW * (g + 1), :]).astype(ml_dtypes.bfloat16),
        })
    return in_maps


def kernel(query, W_qkv, b_qkv, W_out, b_out):
    if "nc" not in _CACHED:
        _CACHED["nc"] = build_nc()
    nc = _CACHED["nc"]
    in_maps = _host_prep(query, W_qkv, b_qkv, W_out, b_out)
    res = run_bass_kernel_spmd(nc, in_maps, core_ids=list(range(NCORES)))
    acc = np.zeros((B, S, D), dtype=np.float64)
    for c, r in enumerate(res.results):
        acc[c // GPB] += np.asarray(r["outp"], dtype=np.float64)
    acc += np.asarray(b_out, dtype=np.float64)[None, None, :]
    return acc.astype(np.float32)
